# revision 29
# baseline (speedup 1.0000x reference)
"""KNN anomaly-score kernel for Trainium2 (8 NeuronCores, Bass/Tile).

Problem: features [B=1024, D=768], memory_bank [N=50000, D=768], k=9.
anomaly_score[b] = mean of the k smallest Euclidean distances from
features[b] to the memory bank rows.

Strategy (per the sharding hint): shard memory-bank rows across the 8
cores.  Each core computes its [B, N/8] block of a selection score
v = f.m - |m|^2/2 + C on the TensorEngine as ONE fp8-e4m3 DoubleRow
GEMM (two K=128 subtiles per instruction, 2x column rate), with the
m-norm folded into the GEMM itself: data dimension D-1 is dropped from
the cross term and its rows repurposed as an augment pair
(features row D-1 := 8.0, bank row D-1 := fp8((C - |m|^2/2)/8),
C = 384).  The per-row |f|^2/2 term is constant along the selection
axis, so it never needs to reach the device - the host adds the exact
x_sq back when converting candidate v values to distances:
d^2 = x_sq + 2C - 2v.

Error budget on v (= -d^2/2 + const, d ~ 39): fp8 rounding of the
cross term ~0.7, the dropped dim-767 cross term ~1.0, fp8 encoding of
the centered m-norm ~0.6 => ~1.4 total, i.e. ~2e-3 relative on d -
well inside the 2e-2 gate.

Selection: for each 1024-column block the DVE MAX8 instruction extracts
the block's top-8 v values straight out of PSUM (no ACT copy).  The
device returns all block candidates [B, 8*nblocks]; the host gathers
the 8 cores' candidates and reduces to the global top-k.  A true top-k
member can be missing only if >=8 elements of its block rank above it,
which forces >=8 of the observed top-k to come from that single block -
the host detects exactly that condition and recomputes the affected
rows with numpy.
"""

import functools
import sys

sys.path.insert(0, "/opt/trn_rl_repo")

import numpy as np

P = 128
NCORES = 8
C_M = 384.0  # centering constant for the fp8 m-norm row: v = f.m + C_M - |m|^2/2


def _ceil_to(x, m):
    return (x + m - 1) // m * m


@functools.lru_cache(maxsize=4)
def _build(B, D, NPAD):
    """Build (and finalize) the SPMD Bass module for one core's shard."""
    from contextlib import ExitStack

    import concourse.tile as tile
    from concourse import bacc, mybir

    f32 = mybir.dt.float32
    bf16 = mybir.dt.bfloat16
    fp8 = mybir.dt.float8e4

    KT = D // P
    MT = B // P
    assert D % P == 0 and B % P == 0 and NPAD >= 1024
    assert KT % 2 == 0, "DoubleRow consumes K=128 subtiles in pairs"
    KP = KT // 2
    # process blocks of 1024 columns (one 2-bank PSUM tile), ragged tail
    chunks = []
    c0 = 0
    while c0 < NPAD:
        w = min(1024, NPAD - c0)
        rem = NPAD - c0 - w
        if 0 < rem < 8:
            w -= 8 - rem  # keep the next (last) chunk MAX8-legal (>=8)
        chunks.append((c0, w))
        c0 += w
    NCH = len(chunks)
    CW = 8 * NCH  # candidates per row per core

    # full 1024-col chunks come from b_main (tile-layout, 6KB/partition
    # contiguous DMA descriptors); the ragged tail from b_tail
    NCHF = sum(1 for _, w in chunks if w == 1024)
    WT = chunks[-1][1] if NCHF < NCH else 0

    nc = bacc.Bacc(
        "TRN2", target_bir_lowering=False, debug=False, num_devices=NCORES
    )

    f_t = nc.declare_dram_parameter("f_t", [P, KT * B], fp8, isOutput=False)
    if NCHF:
        b_main = nc.declare_dram_parameter(
            "b_main", [NCHF * P, KT * 1024], fp8, isOutput=False
        )
    if WT:
        b_tail = nc.declare_dram_parameter("b_tail", [P, KT * WT], fp8, isOutput=False)
    out = nc.declare_dram_parameter("cand", [P, MT * CW], f32, isOutput=True)

    with tile.TileContext(nc) as tc, ExitStack() as ctx:
        cpool = ctx.enter_context(tc.tile_pool(name="const", bufs=1))
        bpool = ctx.enter_context(tc.tile_pool(name="bank", bufs=7))
        ppool = ctx.enter_context(tc.tile_pool(name="psum", bufs=4, space="PSUM"))

        f_view = f_t.rearrange("p (kt b) -> p kt b", kt=KT)
        if NCHF:
            bm_view = b_main.rearrange("(c p) (kt n) -> c p kt n", p=P, kt=KT)
        if WT:
            bt_view = b_tail.rearrange("p (kt n) -> p kt n", kt=KT)

        # PE warm-up during the initial DMA wait: garbage matmuls on a
        # zeroed tile get the HAM clock-gate to 2.4GHz before real work.
        # memset on the gpsimd queue - it is ready ~2us before the vector
        # queue, so warm-up (and thus real work) starts that much earlier.
        warm = cpool.tile([P, 512], bf16, tag="warm")
        nc.gpsimd.memset(warm[:], 0.0)
        wpsum = ppool.tile([P, 1024], f32, tag="pt")  # borrow a pt slot
        for _ in range(8):
            nc.tensor.matmul(
                wpsum[:, :512], lhsT=warm[:, :P], rhs=warm[:], start=True, stop=True
            )

        # chunk 0 + features land first, one full-tile DMA each on the two
        # HWDGE queues: 6KB/partition lines run ~210GB/s vs ~80GB/s for the
        # 2KB lines a kt-pair split would produce.  Later chunks queue up
        # FIFO behind them, so they never compete for HBM with the critical
        # first transfers.
        ftile = cpool.tile([P, KT, B], fp8, tag="ft", name="ft")
        btiles = {}
        btiles[0] = bpool.tile([P, KT, 1024], fp8, tag="bt", name="bt0")
        nc.sync.dma_start(btiles[0][:], bm_view[0])
        nc.scalar.dma_start(ftile[:], f_view[:])
        # only the SP/ACT HWDGE queues move bulk data fast; gpsimd DMA is the
        # slow software-descriptor path
        queues = [nc.sync, nc.scalar]
        for ci in range(1, NCH):
            btile = bpool.tile([P, KT, 1024], fp8, tag="bt", name=f"bt{ci}")
            q = queues[(ci - 1) % len(queues)]
            if ci < NCHF:
                q.dma_start(btile[:], bm_view[ci])
            else:
                q.dma_start(btile[:, :, :WT], bt_view[:])
            btiles[ci] = btile

        # two output tiles so each half's DMA can launch as soon as its last
        # MAX8 lands (parallel queues), instead of one DMA gating on all 56
        MH = MT // 2
        otiles = [
            cpool.tile([P, MH * CW], f32, tag=f"cand{h}", name=f"cand{h}")
            for h in range(2)
        ]

        def chalves(W):
            out_, lo = [], 0
            while lo < W:
                out_.append((lo, min(512, W - lo)))
                lo += 512
            return out_

        for ci, (c0, W) in enumerate(chunks):
            for m in range(MT):
                pt = ppool.tile([P, 1024], f32, tag="pt", name=f"pt{ci}_{m}")
                for j in range(KP):
                    for hlo, hw in chalves(W):
                        nc.tensor.matmul(
                            pt[:, hlo : hlo + hw],
                            lhsT=ftile[:, 2 * j : 2 * j + 2, m * P : (m + 1) * P],
                            rhs=btiles[ci][:, 2 * j : 2 * j + 2, hlo : hlo + hw],
                            start=(j == 0),
                            stop=(j == KP - 1),
                            perf_mode=mybir.MatmulPerfMode.DoubleRow,
                        )
                h, mh = divmod(m, MH)
                nc.vector.max(
                    otiles[h][:, mh * CW + ci * 8 : mh * CW + ci * 8 + 8],
                    pt[:, :W],
                )

        nc.sync.dma_start(out[:, : MH * CW], otiles[0][:])
        nc.scalar.dma_start(out[:, MH * CW :], otiles[1][:])

    nc.finalize()
    return nc


def _tile_f(fT, KT):
    """[D, B] -> tile layout [P, KT*B] (d = kt*P + p)."""
    D, B = fT.shape
    return np.ascontiguousarray(
        fT.reshape(KT, P, B).transpose(1, 0, 2).reshape(P, KT * B)
    )


def _tile_b(bT, KT, NCHF, WT):
    """[D, NPAD] -> (b_main [NCHF*P, KT*1024], b_tail [P, KT*WT] | None)."""
    D = bT.shape[0]
    bm = bT[:, : NCHF * 1024].reshape(KT, P, NCHF, 1024)
    b_main = np.ascontiguousarray(
        bm.transpose(2, 1, 0, 3).reshape(NCHF * P, KT * 1024)
    )
    b_tail = None
    if WT:
        tl = bT[:, NCHF * 1024 :].reshape(KT, P, WT)
        b_tail = np.ascontiguousarray(tl.transpose(1, 0, 2).reshape(P, KT * WT))
    return b_main, b_tail


def _shard_geom(N):
    NSH = -(-N // NCORES)
    NPAD = max(NSH, 1024)
    if NPAD % 1024 and NPAD % 1024 < 8:
        NPAD = _ceil_to(NPAD, 1024)  # keep the ragged tail MAX8-legal (>=8)
    NCHF = NPAD // 1024
    WT = NPAD - NCHF * 1024
    return NSH, NPAD, NCHF, WT


def _host_prep(features, memory_bank):
    """Shard + lay out inputs for the 8 cores."""
    import ml_dtypes

    f8 = ml_dtypes.float8_e4m3
    B, D = features.shape
    N = memory_bank.shape[0]
    NSH, NPAD, NCHF, WT = _shard_geom(N)
    KT = D // P

    # Cross term uses data dims 0..D-2 only; row D-1 carries the m-norm:
    #   v = f[:D-1].m[:D-1] + 8 * fp8((C_M - |m|^2/2) / 8)
    fT = np.ascontiguousarray(features.T).astype(f8)
    fT[D - 1] = f8(8.0)
    x_sq = np.einsum("bd,bd->b", features, features, dtype=np.float32)
    f_tiled = _tile_f(fT, KT)

    msq = np.einsum("nd,nd->n", memory_bank, memory_bank, dtype=np.float32)

    in_maps = []
    for i in range(NCORES):
        lo = i * NSH
        hi = min(lo + NSH, N)
        n_i = hi - lo
        bT = np.zeros((D, NPAD), f8)
        bT[:, :n_i] = memory_bank[lo:hi].T.astype(f8)
        q_m = np.full(NPAD, -240.0, np.float32)  # pads: v = -1920, never top-8
        q_m[:n_i] = (C_M - 0.5 * msq[lo:hi]) / 8.0
        bT[D - 1] = q_m.astype(f8)
        b_main, b_tail = _tile_b(bT, KT, NCHF, WT)
        im = {"f_t": f_tiled, "b_main": b_main}
        if b_tail is not None:
            im["b_tail"] = b_tail
        in_maps.append(im)
    return in_maps, NPAD, x_sq, msq


# test.py can flip these to get a profiled run
TRACE = False
LAST_RESULT = None
N_RECOMPUTED = 0


def _install_ntff_hook():
    """This container's `antenv` lacks `axon_hooks`; synthesize it so
    run_bass_kernel_spmd(trace=True) can profile via the axon .so."""
    import sys as _sys

    if "antenv.axon_hooks" in _sys.modules:
        return
    import contextlib, ctypes, types

    mod = types.ModuleType("antenv.axon_hooks")
    mod._hook = None
    mod.set_axon_ntff_profile_hook = lambda h: setattr(mod, "_hook", h)
    mod.get_axon_ntff_profile_hook = lambda: mod._hook

    so_path = "/opt/axon/libaxon_pjrt.so"
    try:
        lib = ctypes.CDLL(so_path)
        lib.axon_start_nrt_profile.argtypes = [
            ctypes.POINTER(ctypes.c_int64),
            ctypes.c_size_t,
        ]
        lib.axon_start_nrt_profile.restype = ctypes.c_int64
        lib.axon_stop_nrt_profile.argtypes = [ctypes.c_char_p]
        lib.axon_stop_nrt_profile.restype = ctypes.c_int64

        @contextlib.contextmanager
        def _hook(output_dir, device_ids):
            import jax

            jax.devices()
            if device_ids:
                ids = (ctypes.c_int64 * len(device_ids))(*device_ids)
                rc = lib.axon_start_nrt_profile(ids, len(device_ids))
            else:
                rc = lib.axon_start_nrt_profile(None, 0)
            if rc != 0:
                raise RuntimeError(f"axon_start_nrt_profile rc={rc}")
            try:
                yield
            finally:
                n = lib.axon_stop_nrt_profile(str(output_dir).encode())
                print(f"profile: {n} file(s) written to {output_dir}")

        mod._hook = _hook
    except (OSError, AttributeError):
        pass

    import antenv

    _sys.modules["antenv.axon_hooks"] = mod
    antenv.axon_hooks = mod


def _exact_row_scores(features, memory_bank, rows, kk):
    """Exact numpy top-k mean distance for a few suspect rows."""
    f = features[rows]  # [R, D]
    d2 = (
        np.einsum("rd,rd->r", f, f)[:, None]
        + np.einsum("nd,nd->n", memory_bank, memory_bank)[None, :]
        - 2.0 * (f @ memory_bank.T)
    )
    d2k = np.sort(d2, axis=1)[:, :kk]
    return np.sqrt(np.maximum(d2k, 0.0)).mean(axis=1)


def kernel(features, memory_bank, k):
    global LAST_RESULT, N_RECOMPUTED
    from concourse.bass_utils import run_bass_kernel_spmd

    features = np.asarray(features, dtype=np.float32)
    memory_bank = np.asarray(memory_bank, dtype=np.float32)
    B, D = features.shape
    N = memory_bank.shape[0]
    kk = min(int(k), N)
    if kk <= 0:
        # mean over an empty candidate set (matches jnp.mean of empty)
        return np.full(B, np.nan, np.float32)

    in_maps, NPAD, x_sq, msq = _host_prep(features, memory_bank)
    nc = _build(B, D, NPAD)

    if TRACE:
        _install_ntff_hook()
    res = run_bass_kernel_spmd(nc, in_maps, list(range(NCORES)), trace=TRACE)
    LAST_RESULT = res

    # gather per-(core, block) top-8 candidates; larger v = closer
    MT = B // P
    v = np.concatenate(
        [_untile_cand(res.results[i]["cand"], MT) for i in range(NCORES)], axis=1
    )  # [B, NCORES * 8 * nblocks]
    return _finalize(v, x_sq, features, memory_bank, kk)


def _untile_cand(arr, MT):
    """Device cand layout [P, MT*CW] -> [B, CW] (b = m*P + p)."""
    CW = arr.shape[1] // MT
    return arr.reshape(P, MT, CW).transpose(1, 0, 2).reshape(MT * P, CW)


def _finalize(v, x_sq, features, memory_bank, kk):
    """Reduce the per-(core, block) top-8 candidates to the final scores."""
    global N_RECOMPUTED
    kk_c = min(kk, v.shape[1])
    order = np.argsort(-v, axis=1)[:, :kk_c]  # observed top-k candidates
    vk = np.take_along_axis(v, order, axis=1)
    # v = f.m_trunc + C_M - |m|^2/2  =>  d^2 = x_sq + 2*C_M - 2*v
    d = np.sqrt(np.maximum(x_sq[:, None] + 2.0 * C_M - 2.0 * vk, 0.0))
    scores = d.mean(axis=1).astype(np.float32)

    # A true top-k member can only be missing if >=8 elements of its
    # 1024-column block outrank it; then >=8 of the observed top-k come
    # from that block (index group of 8).  Recompute such rows exactly.
    N_RECOMPUTED = 0
    if kk >= 9:
        if kk > v.shape[1]:  # more than the candidate pool: all rows exact
            suspects = np.arange(v.shape[0])
        else:
            grp = np.sort(order // 8, axis=1)
            same8 = (grp[:, 7:] == grp[:, : grp.shape[1] - 7]).any(axis=1)
            suspects = np.nonzero(same8)[0]
        if suspects.size:
            N_RECOMPUTED = suspects.size
            scores[suspects] = _exact_row_scores(
                features, memory_bank, suspects, kk
            ).astype(np.float32)

    return scores



# revision 30
# speedup vs baseline: 1.0157x; 1.0157x over previous
"""KNN anomaly-score kernel for Trainium2 (8 NeuronCores, Bass/Tile).

Problem: features [B=1024, D=768], memory_bank [N=50000, D=768], k=9.
anomaly_score[b] = mean of the k smallest Euclidean distances from
features[b] to the memory bank rows.

Strategy (per the sharding hint): shard memory-bank rows across the 8
cores.  Each core computes its [B, N/8] block of a selection score
v = f.m - |m|^2/2 + C on the TensorEngine as ONE fp8-e4m3 DoubleRow
GEMM (two K=128 subtiles per instruction, 2x column rate), with the
m-norm folded into the GEMM itself: data dimension D-1 is dropped from
the cross term and its rows repurposed as an augment pair
(features row D-1 := 8.0, bank row D-1 := fp8((C - |m|^2/2)/8),
C = 384).  The per-row |f|^2/2 term is constant along the selection
axis, so it never needs to reach the device - the host adds the exact
x_sq back when converting candidate v values to distances:
d^2 = x_sq + 2C - 2v.

Error budget on v (= -d^2/2 + const, d ~ 39): fp8 rounding of the
cross term ~0.7, the dropped dim-767 cross term ~1.0, fp8 encoding of
the centered m-norm ~0.6 => ~1.4 total, i.e. ~2e-3 relative on d -
well inside the 2e-2 gate.

Selection: for each 1024-column block the DVE MAX8 instruction extracts
the block's top-8 v values straight out of PSUM (no ACT copy).  The
device returns all block candidates [B, 8*nblocks]; the host gathers
the 8 cores' candidates and reduces to the global top-k.  A true top-k
member can be missing only if >=8 elements of its block rank above it,
which forces >=8 of the observed top-k to come from that single block -
the host detects exactly that condition and recomputes the affected
rows with numpy.
"""

import functools
import sys

sys.path.insert(0, "/opt/trn_rl_repo")

import numpy as np

P = 128
NCORES = 8
C_M = 384.0  # centering constant for the fp8 m-norm row: v = f.m + C_M - |m|^2/2


def _ceil_to(x, m):
    return (x + m - 1) // m * m


@functools.lru_cache(maxsize=4)
def _build(B, D, NPAD):
    """Build (and finalize) the SPMD Bass module for one core's shard."""
    from contextlib import ExitStack

    import concourse.tile as tile
    from concourse import bacc, mybir

    f32 = mybir.dt.float32
    bf16 = mybir.dt.bfloat16
    fp8 = mybir.dt.float8e4

    KT = D // P
    MT = B // P
    assert D % P == 0 and B % P == 0 and NPAD >= 1024
    assert KT % 2 == 0, "DoubleRow consumes K=128 subtiles in pairs"
    KP = KT // 2
    # process blocks of 1024 columns (one 2-bank PSUM tile), ragged tail
    chunks = []
    c0 = 0
    while c0 < NPAD:
        w = min(1024, NPAD - c0)
        rem = NPAD - c0 - w
        if 0 < rem < 8:
            w -= 8 - rem  # keep the next (last) chunk MAX8-legal (>=8)
        chunks.append((c0, w))
        c0 += w
    NCH = len(chunks)
    CW = 8 * NCH  # candidates per row per core

    # full 1024-col chunks come from b_main (tile-layout, 6KB/partition
    # contiguous DMA descriptors); the ragged tail from b_tail
    NCHF = sum(1 for _, w in chunks if w == 1024)
    WT = chunks[-1][1] if NCHF < NCH else 0

    nc = bacc.Bacc(
        "TRN2", target_bir_lowering=False, debug=False, num_devices=NCORES
    )

    f_t = nc.declare_dram_parameter("f_t", [P, KT * B], fp8, isOutput=False)
    if NCHF:
        b_main = nc.declare_dram_parameter(
            "b_main", [NCHF * P, KT * 1024], fp8, isOutput=False
        )
    if WT:
        b_tail = nc.declare_dram_parameter("b_tail", [P, KT * WT], fp8, isOutput=False)
    out = nc.declare_dram_parameter("cand", [P, MT * CW], f32, isOutput=True)

    with tile.TileContext(nc) as tc, ExitStack() as ctx:
        cpool = ctx.enter_context(tc.tile_pool(name="const", bufs=1))
        bpool = ctx.enter_context(tc.tile_pool(name="bank", bufs=7))
        ppool = ctx.enter_context(tc.tile_pool(name="psum", bufs=4, space="PSUM"))

        f_view = f_t.rearrange("p (kt b) -> p kt b", kt=KT)
        if NCHF:
            bm_view = b_main.rearrange("(c p) (kt n) -> c p kt n", p=P, kt=KT)
        if WT:
            bt_view = b_tail.rearrange("p (kt n) -> p kt n", kt=KT)

        # PE warm-up during the initial DMA wait: garbage matmuls on a
        # zeroed tile get the HAM clock-gate to 2.4GHz before real work.
        # memset on the gpsimd queue - it is ready ~2us before the vector
        # queue, so warm-up (and thus real work) starts that much earlier.
        warm = cpool.tile([P, 512], bf16, tag="warm")
        nc.gpsimd.memset(warm[:], 0.0)
        wpsum = ppool.tile([P, 1024], f32, tag="pt")  # borrow a pt slot
        # warm matmuls run ~427ns apart at the ramping clock; 17 of them
        # cover the ~7.5us the first-wave DMAs (bt0 + ftile) need to land,
        # so the PE never idles (an idle gap resets the p-state ramp)
        for _ in range(17):
            nc.tensor.matmul(
                wpsum[:, :512], lhsT=warm[:, :P], rhs=warm[:], start=True, stop=True
            )

        # chunk 0 + features land first, one full-tile DMA each on the two
        # HWDGE queues: 6KB/partition lines run ~210GB/s vs ~80GB/s for the
        # 2KB lines a kt-pair split would produce.  Later chunks queue up
        # FIFO behind them, so they never compete for HBM with the critical
        # first transfers.
        ftile = cpool.tile([P, KT, B], fp8, tag="ft", name="ft")
        btiles = {}
        btiles[0] = bpool.tile([P, KT, 1024], fp8, tag="bt", name="bt0")
        nc.sync.dma_start(btiles[0][:], bm_view[0])
        nc.scalar.dma_start(ftile[:], f_view[:])
        # only the SP/ACT HWDGE queues move bulk data fast; gpsimd DMA is the
        # slow software-descriptor path
        queues = [nc.sync, nc.scalar]
        for ci in range(1, NCH):
            btile = bpool.tile([P, KT, 1024], fp8, tag="bt", name=f"bt{ci}")
            q = queues[(ci - 1) % len(queues)]
            if ci < NCHF:
                q.dma_start(btile[:], bm_view[ci])
            else:
                q.dma_start(btile[:, :, :WT], bt_view[:])
            btiles[ci] = btile

        # two output tiles so each half's DMA can launch as soon as its last
        # MAX8 lands (parallel queues), instead of one DMA gating on all 56
        MH = MT // 2
        otiles = [
            cpool.tile([P, MH * CW], f32, tag=f"cand{h}", name=f"cand{h}")
            for h in range(2)
        ]

        def chalves(W):
            out_, lo = [], 0
            while lo < W:
                out_.append((lo, min(512, W - lo)))
                lo += 512
            return out_

        for ci, (c0, W) in enumerate(chunks):
            for m in range(MT):
                pt = ppool.tile([P, 1024], f32, tag="pt", name=f"pt{ci}_{m}")
                for j in range(KP):
                    for hlo, hw in chalves(W):
                        nc.tensor.matmul(
                            pt[:, hlo : hlo + hw],
                            lhsT=ftile[:, 2 * j : 2 * j + 2, m * P : (m + 1) * P],
                            rhs=btiles[ci][:, 2 * j : 2 * j + 2, hlo : hlo + hw],
                            start=(j == 0),
                            stop=(j == KP - 1),
                            perf_mode=mybir.MatmulPerfMode.DoubleRow,
                        )
                h, mh = divmod(m, MH)
                nc.vector.max(
                    otiles[h][:, mh * CW + ci * 8 : mh * CW + ci * 8 + 8],
                    pt[:, :W],
                )

        nc.sync.dma_start(out[:, : MH * CW], otiles[0][:])
        nc.scalar.dma_start(out[:, MH * CW :], otiles[1][:])

    nc.finalize()
    return nc


def _tile_f(fT, KT):
    """[D, B] -> tile layout [P, KT*B] (d = kt*P + p)."""
    D, B = fT.shape
    return np.ascontiguousarray(
        fT.reshape(KT, P, B).transpose(1, 0, 2).reshape(P, KT * B)
    )


def _tile_b(bT, KT, NCHF, WT):
    """[D, NPAD] -> (b_main [NCHF*P, KT*1024], b_tail [P, KT*WT] | None)."""
    D = bT.shape[0]
    bm = bT[:, : NCHF * 1024].reshape(KT, P, NCHF, 1024)
    b_main = np.ascontiguousarray(
        bm.transpose(2, 1, 0, 3).reshape(NCHF * P, KT * 1024)
    )
    b_tail = None
    if WT:
        tl = bT[:, NCHF * 1024 :].reshape(KT, P, WT)
        b_tail = np.ascontiguousarray(tl.transpose(1, 0, 2).reshape(P, KT * WT))
    return b_main, b_tail


def _shard_geom(N):
    NSH = -(-N // NCORES)
    NPAD = max(NSH, 1024)
    if NPAD % 1024 and NPAD % 1024 < 8:
        NPAD = _ceil_to(NPAD, 1024)  # keep the ragged tail MAX8-legal (>=8)
    NCHF = NPAD // 1024
    WT = NPAD - NCHF * 1024
    return NSH, NPAD, NCHF, WT


def _host_prep(features, memory_bank):
    """Shard + lay out inputs for the 8 cores."""
    import ml_dtypes

    f8 = ml_dtypes.float8_e4m3
    B, D = features.shape
    N = memory_bank.shape[0]
    NSH, NPAD, NCHF, WT = _shard_geom(N)
    KT = D // P

    # Cross term uses data dims 0..D-2 only; row D-1 carries the m-norm:
    #   v = f[:D-1].m[:D-1] + 8 * fp8((C_M - |m|^2/2) / 8)
    fT = np.ascontiguousarray(features.T).astype(f8)
    fT[D - 1] = f8(8.0)
    x_sq = np.einsum("bd,bd->b", features, features, dtype=np.float32)
    f_tiled = _tile_f(fT, KT)

    msq = np.einsum("nd,nd->n", memory_bank, memory_bank, dtype=np.float32)

    in_maps = []
    for i in range(NCORES):
        lo = i * NSH
        hi = min(lo + NSH, N)
        n_i = hi - lo
        bT = np.zeros((D, NPAD), f8)
        bT[:, :n_i] = memory_bank[lo:hi].T.astype(f8)
        q_m = np.full(NPAD, -240.0, np.float32)  # pads: v = -1920, never top-8
        q_m[:n_i] = (C_M - 0.5 * msq[lo:hi]) / 8.0
        bT[D - 1] = q_m.astype(f8)
        b_main, b_tail = _tile_b(bT, KT, NCHF, WT)
        im = {"f_t": f_tiled, "b_main": b_main}
        if b_tail is not None:
            im["b_tail"] = b_tail
        in_maps.append(im)
    return in_maps, NPAD, x_sq, msq


# test.py can flip these to get a profiled run
TRACE = False
LAST_RESULT = None
N_RECOMPUTED = 0


def _install_ntff_hook():
    """This container's `antenv` lacks `axon_hooks`; synthesize it so
    run_bass_kernel_spmd(trace=True) can profile via the axon .so."""
    import sys as _sys

    if "antenv.axon_hooks" in _sys.modules:
        return
    import contextlib, ctypes, types

    mod = types.ModuleType("antenv.axon_hooks")
    mod._hook = None
    mod.set_axon_ntff_profile_hook = lambda h: setattr(mod, "_hook", h)
    mod.get_axon_ntff_profile_hook = lambda: mod._hook

    so_path = "/opt/axon/libaxon_pjrt.so"
    try:
        lib = ctypes.CDLL(so_path)
        lib.axon_start_nrt_profile.argtypes = [
            ctypes.POINTER(ctypes.c_int64),
            ctypes.c_size_t,
        ]
        lib.axon_start_nrt_profile.restype = ctypes.c_int64
        lib.axon_stop_nrt_profile.argtypes = [ctypes.c_char_p]
        lib.axon_stop_nrt_profile.restype = ctypes.c_int64

        @contextlib.contextmanager
        def _hook(output_dir, device_ids):
            import jax

            jax.devices()
            if device_ids:
                ids = (ctypes.c_int64 * len(device_ids))(*device_ids)
                rc = lib.axon_start_nrt_profile(ids, len(device_ids))
            else:
                rc = lib.axon_start_nrt_profile(None, 0)
            if rc != 0:
                raise RuntimeError(f"axon_start_nrt_profile rc={rc}")
            try:
                yield
            finally:
                n = lib.axon_stop_nrt_profile(str(output_dir).encode())
                print(f"profile: {n} file(s) written to {output_dir}")

        mod._hook = _hook
    except (OSError, AttributeError):
        pass

    import antenv

    _sys.modules["antenv.axon_hooks"] = mod
    antenv.axon_hooks = mod


def _exact_row_scores(features, memory_bank, rows, kk):
    """Exact numpy top-k mean distance for a few suspect rows."""
    f = features[rows]  # [R, D]
    d2 = (
        np.einsum("rd,rd->r", f, f)[:, None]
        + np.einsum("nd,nd->n", memory_bank, memory_bank)[None, :]
        - 2.0 * (f @ memory_bank.T)
    )
    d2k = np.sort(d2, axis=1)[:, :kk]
    return np.sqrt(np.maximum(d2k, 0.0)).mean(axis=1)


def kernel(features, memory_bank, k):
    global LAST_RESULT, N_RECOMPUTED
    from concourse.bass_utils import run_bass_kernel_spmd

    features = np.asarray(features, dtype=np.float32)
    memory_bank = np.asarray(memory_bank, dtype=np.float32)
    B, D = features.shape
    N = memory_bank.shape[0]
    kk = min(int(k), N)
    if kk <= 0:
        # mean over an empty candidate set (matches jnp.mean of empty)
        return np.full(B, np.nan, np.float32)

    in_maps, NPAD, x_sq, msq = _host_prep(features, memory_bank)
    nc = _build(B, D, NPAD)

    if TRACE:
        _install_ntff_hook()
    res = run_bass_kernel_spmd(nc, in_maps, list(range(NCORES)), trace=TRACE)
    LAST_RESULT = res

    # gather per-(core, block) top-8 candidates; larger v = closer
    MT = B // P
    v = np.concatenate(
        [_untile_cand(res.results[i]["cand"], MT) for i in range(NCORES)], axis=1
    )  # [B, NCORES * 8 * nblocks]
    return _finalize(v, x_sq, features, memory_bank, kk)


def _untile_cand(arr, MT):
    """Device cand layout [P, MT*CW] -> [B, CW] (b = m*P + p)."""
    CW = arr.shape[1] // MT
    return arr.reshape(P, MT, CW).transpose(1, 0, 2).reshape(MT * P, CW)


def _finalize(v, x_sq, features, memory_bank, kk):
    """Reduce the per-(core, block) top-8 candidates to the final scores."""
    global N_RECOMPUTED
    kk_c = min(kk, v.shape[1])
    order = np.argsort(-v, axis=1)[:, :kk_c]  # observed top-k candidates
    vk = np.take_along_axis(v, order, axis=1)
    # v = f.m_trunc + C_M - |m|^2/2  =>  d^2 = x_sq + 2*C_M - 2*v
    d = np.sqrt(np.maximum(x_sq[:, None] + 2.0 * C_M - 2.0 * vk, 0.0))
    scores = d.mean(axis=1).astype(np.float32)

    # A true top-k member can only be missing if >=8 elements of its
    # 1024-column block outrank it; then >=8 of the observed top-k come
    # from that block (index group of 8).  Recompute such rows exactly.
    N_RECOMPUTED = 0
    if kk >= 9:
        if kk > v.shape[1]:  # more than the candidate pool: all rows exact
            suspects = np.arange(v.shape[0])
        else:
            grp = np.sort(order // 8, axis=1)
            same8 = (grp[:, 7:] == grp[:, : grp.shape[1] - 7]).any(axis=1)
            suspects = np.nonzero(same8)[0]
        if suspects.size:
            N_RECOMPUTED = suspects.size
            scores[suspects] = _exact_row_scores(
                features, memory_bank, suspects, kk
            ).astype(np.float32)

    return scores



# revision 38
# speedup vs baseline: 1.0396x; 1.0235x over previous
"""KNN anomaly-score kernel for Trainium2 (8 NeuronCores, Bass/Tile).

Problem: features [B=1024, D=768], memory_bank [N=50000, D=768], k=9.
anomaly_score[b] = mean of the k smallest Euclidean distances from
features[b] to the memory bank rows.

Strategy (per the sharding hint): shard memory-bank rows across the 8
cores.  Each core computes its [B, N/8] block of a selection score
v = f.m - |m|^2/2 + C on the TensorEngine as ONE fp8-e4m3 DoubleRow
GEMM (two K=128 subtiles per instruction, 2x column rate), with the
m-norm folded into the GEMM itself: data dimension D-1 is dropped from
the cross term and its rows repurposed as an augment pair
(features row D-1 := 8.0, bank row D-1 := fp8((C - |m|^2/2)/8),
C = 384).  The per-row |f|^2/2 term is constant along the selection
axis, so it never needs to reach the device - the host adds the exact
x_sq back when converting candidate v values to distances:
d^2 = x_sq + 2C - 2v.

Error budget on v (= -d^2/2 + const, d ~ 39): fp8 rounding of the
cross term ~0.7, the dropped dim-767 cross term ~1.0, fp8 encoding of
the centered m-norm ~0.6 => ~1.4 total, i.e. ~2e-3 relative on d -
well inside the 2e-2 gate.

Selection: for each 1024-column block the DVE MAX8 instruction extracts
the block's top-8 v values straight out of PSUM (no ACT copy).  The
device returns all block candidates [B, 8*nblocks]; the host gathers
the 8 cores' candidates and reduces to the global top-k.  A true top-k
member can be missing only if >=8 elements of its block rank above it,
which forces >=8 of the observed top-k to come from that single block -
the host detects exactly that condition and recomputes the affected
rows with numpy.
"""

import functools
import sys

sys.path.insert(0, "/opt/trn_rl_repo")

import numpy as np

P = 128
NCORES = 8
C_M = 384.0  # centering constant for the fp8 m-norm row: v = f.m + C_M - |m|^2/2


def _ceil_to(x, m):
    return (x + m - 1) // m * m


@functools.lru_cache(maxsize=4)
def _build(B, D, NPAD):
    """Build (and finalize) the SPMD Bass module for one core's shard."""
    from contextlib import ExitStack

    import concourse.tile as tile
    from concourse import bacc, mybir

    f32 = mybir.dt.float32
    bf16 = mybir.dt.bfloat16
    fp8 = mybir.dt.float8e4

    KT = D // P
    MT = B // P
    assert D % P == 0 and B % P == 0 and NPAD >= 1024
    assert KT % 2 == 0, "DoubleRow consumes K=128 subtiles in pairs"
    KP = KT // 2
    widths = _chunk_plan(NPAD)
    chunks = []
    c0 = 0
    for w in widths:
        chunks.append((c0, w))
        c0 += w
    NCH = len(chunks)
    CW = 8 * NCH  # candidates per row per core

    # chunk 0 is half-width so the gating first-wave DMA is small; full
    # 1024-col chunks come from b_main (tile-layout, 6KB/partition
    # contiguous DMA descriptors); the ragged tail from b_tail
    W0 = widths[0]
    NCHF = sum(1 for w in widths if w == 1024)
    WT = widths[-1] if widths[-1] != 1024 and len(widths) > 1 else 0

    nc = bacc.Bacc(
        "TRN2", target_bir_lowering=False, debug=False, num_devices=NCORES
    )

    f_t = nc.declare_dram_parameter("f_t", [P, KT * B], fp8, isOutput=False)
    b_c0 = nc.declare_dram_parameter("b_c0", [P, KT * W0], fp8, isOutput=False)
    if NCHF:
        b_main = nc.declare_dram_parameter(
            "b_main", [NCHF * P, KT * 1024], fp8, isOutput=False
        )
    if WT:
        b_tail = nc.declare_dram_parameter("b_tail", [P, KT * WT], fp8, isOutput=False)
    out = nc.declare_dram_parameter("cand", [P, MT * CW], f32, isOutput=True)

    with tile.TileContext(nc) as tc, ExitStack() as ctx:
        cpool = ctx.enter_context(tc.tile_pool(name="const", bufs=1))
        bpool = ctx.enter_context(tc.tile_pool(name="bank", bufs=7))
        ppool = ctx.enter_context(tc.tile_pool(name="psum", bufs=4, space="PSUM"))

        f_view = f_t.rearrange("p (kt b) -> p kt b", kt=KT)
        bc0_view = b_c0.rearrange("p (kt n) -> p kt n", kt=KT)
        if NCHF:
            bm_view = b_main.rearrange("(c p) (kt n) -> c p kt n", p=P, kt=KT)
        if WT:
            bt_view = b_tail.rearrange("p (kt n) -> p kt n", kt=KT)

        # PE warm-up during the initial DMA wait: garbage matmuls on a
        # zeroed tile get the HAM clock-gate to 2.4GHz before real work.
        # memset on the gpsimd queue - it is ready ~2us before the vector
        # queue, so warm-up (and thus real work) starts that much earlier.
        warm = cpool.tile([P, 512], bf16, tag="warm")
        nc.gpsimd.memset(warm[:], 0.0)
        wpsum = ppool.tile([P, 1024], f32, tag="pt")  # borrow a pt slot
        # warm matmuls run ~427ns apart at the ramping clock; cover the
        # ~5us the first-wave DMAs need to land, so the PE never idles
        # (an idle gap resets the p-state ramp)
        for _ in range(11):
            nc.tensor.matmul(
                wpsum[:, :512], lhsT=warm[:, :P], rhs=warm[:], start=True, stop=True
            )

        # first wave: half-width chunk 0 on sync, features split across
        # scalar + gpsimd so all three DMA engines stream in parallel
        # (~100GB/s per HWDGE queue, ~65GB/s for the gpsimd SWDGE path).
        # Later chunks queue FIFO behind them, so they never compete for
        # HBM with the critical first transfers.
        ftile = cpool.tile([P, KT, B], fp8, tag="ft", name="ft")
        btiles = {}
        btiles[0] = bpool.tile([P, KT, 1024], fp8, tag="bt", name="bt0")
        nc.sync.dma_start(btiles[0][:, :, :W0], bc0_view[:])
        KA = min(4, KT)
        nc.scalar.dma_start(ftile[:, :KA, :], f_view[:, :KA, :])
        if KT > KA:
            nc.gpsimd.dma_start(ftile[:, KA:, :], f_view[:, KA:, :])
        queues = [nc.sync, nc.scalar]
        for ci in range(1, NCH):
            btile = bpool.tile([P, KT, 1024], fp8, tag="bt", name=f"bt{ci}")
            q = queues[(ci - 1) % len(queues)]
            if ci <= NCHF:
                q.dma_start(btile[:], bm_view[ci - 1])
            else:
                q.dma_start(btile[:, :, :WT], bt_view[:])
            btiles[ci] = btile

        # two output tiles so each half's DMA can launch as soon as its last
        # MAX8 lands (parallel queues), instead of one DMA gating on all 56
        MH = MT // 2
        otiles = [
            cpool.tile([P, MH * CW], f32, tag=f"cand{h}", name=f"cand{h}")
            for h in range(2)
        ]

        def chalves(W):
            out_, lo = [], 0
            while lo < W:
                out_.append((lo, min(512, W - lo)))
                lo += 512
            return out_

        for ci, (c0, W) in enumerate(chunks):
            for m in range(MT):
                pt = ppool.tile([P, 1024], f32, tag="pt", name=f"pt{ci}_{m}")
                for j in range(KP):
                    for hlo, hw in chalves(W):
                        nc.tensor.matmul(
                            pt[:, hlo : hlo + hw],
                            lhsT=ftile[:, 2 * j : 2 * j + 2, m * P : (m + 1) * P],
                            rhs=btiles[ci][:, 2 * j : 2 * j + 2, hlo : hlo + hw],
                            start=(j == 0),
                            stop=(j == KP - 1),
                            perf_mode=mybir.MatmulPerfMode.DoubleRow,
                        )
                h, mh = divmod(m, MH)
                nc.vector.max(
                    otiles[h][:, mh * CW + ci * 8 : mh * CW + ci * 8 + 8],
                    pt[:, :W],
                )

        nc.sync.dma_start(out[:, : MH * CW], otiles[0][:])
        nc.scalar.dma_start(out[:, MH * CW :], otiles[1][:])

    nc.finalize()
    return nc


def _chunk_plan(NPAD):
    """Chunk widths: a 512 starter (small gating first-wave DMA), full
    1024s, then a ragged tail (kept >=8 so MAX8 stays legal)."""
    if NPAD <= 1024:
        return [NPAD]
    w0 = 512
    nf = (NPAD - w0) // 1024
    tail = NPAD - w0 - nf * 1024
    if 0 < tail < 8:
        w0 -= 8 - tail
        tail = 8
    return [w0] + [1024] * nf + ([tail] if tail else [])


def _seg(bT, KT, off, W):
    """Columns [off, off+W) of [D, cols] -> tile layout [P, KT*W]."""
    D = bT.shape[0]
    P_ = P
    s = bT[:, off : off + W].reshape(KT, P_, W)
    return np.ascontiguousarray(s.transpose(1, 0, 2).reshape(P_, KT * W))


def _tile_f(fT, KT):
    """[D, B] -> tile layout [P, KT*B] (d = kt*P + p)."""
    return _seg(fT, KT, 0, fT.shape[1])


def _unseg(arr, KT, W):
    """Tile layout [P, KT*W] -> [D, W] (inverse of _seg, for the sim check)."""
    return arr.reshape(P, KT, W).transpose(1, 0, 2).reshape(KT * P, W)


def _tile_b(bT, KT, widths):
    """[D, NPAD] -> dict of b_c0 / b_main / b_tail tile-layout arrays."""
    W0 = widths[0]
    nf = sum(1 for w in widths if w == 1024)
    WT = widths[-1] if len(widths) > 1 and widths[-1] != 1024 else 0
    out = {"b_c0": _seg(bT, KT, 0, W0)}
    if nf:
        out["b_main"] = np.concatenate(
            [_seg(bT, KT, W0 + i * 1024, 1024) for i in range(nf)], axis=0
        )
    if WT:
        out["b_tail"] = _seg(bT, KT, W0 + nf * 1024, WT)
    return out


def _shard_geom(N):
    NSH = -(-N // NCORES)
    NPAD = max(NSH, 1024)
    return NSH, NPAD, _chunk_plan(NPAD)


def _host_prep(features, memory_bank):
    """Shard + lay out inputs for the 8 cores."""
    import ml_dtypes

    f8 = ml_dtypes.float8_e4m3
    B, D = features.shape
    N = memory_bank.shape[0]
    NSH, NPAD, widths = _shard_geom(N)
    KT = D // P

    # Cross term uses data dims 0..D-2 only; row D-1 carries the m-norm:
    #   v = f[:D-1].m[:D-1] + 8 * fp8((C_M - |m|^2/2) / 8)
    fT = np.ascontiguousarray(features.T).astype(f8)
    fT[D - 1] = f8(8.0)
    x_sq = np.einsum("bd,bd->b", features, features, dtype=np.float32)
    f_tiled = _tile_f(fT, KT)

    msq = np.einsum("nd,nd->n", memory_bank, memory_bank, dtype=np.float32)

    in_maps = []
    for i in range(NCORES):
        lo = i * NSH
        hi = min(lo + NSH, N)
        n_i = hi - lo
        bT = np.zeros((D, NPAD), f8)
        bT[:, :n_i] = memory_bank[lo:hi].T.astype(f8)
        q_m = np.full(NPAD, -240.0, np.float32)  # pads: v = -1920, never top-8
        q_m[:n_i] = (C_M - 0.5 * msq[lo:hi]) / 8.0
        bT[D - 1] = q_m.astype(f8)
        im = {"f_t": f_tiled}
        im.update(_tile_b(bT, KT, widths))
        in_maps.append(im)
    return in_maps, NPAD, x_sq, msq


# test.py can flip these to get a profiled run
TRACE = False
LAST_RESULT = None
N_RECOMPUTED = 0


def _install_ntff_hook():
    """This container's `antenv` lacks `axon_hooks`; synthesize it so
    run_bass_kernel_spmd(trace=True) can profile via the axon .so."""
    import sys as _sys

    if "antenv.axon_hooks" in _sys.modules:
        return
    import contextlib, ctypes, types

    mod = types.ModuleType("antenv.axon_hooks")
    mod._hook = None
    mod.set_axon_ntff_profile_hook = lambda h: setattr(mod, "_hook", h)
    mod.get_axon_ntff_profile_hook = lambda: mod._hook

    so_path = "/opt/axon/libaxon_pjrt.so"
    try:
        lib = ctypes.CDLL(so_path)
        lib.axon_start_nrt_profile.argtypes = [
            ctypes.POINTER(ctypes.c_int64),
            ctypes.c_size_t,
        ]
        lib.axon_start_nrt_profile.restype = ctypes.c_int64
        lib.axon_stop_nrt_profile.argtypes = [ctypes.c_char_p]
        lib.axon_stop_nrt_profile.restype = ctypes.c_int64

        @contextlib.contextmanager
        def _hook(output_dir, device_ids):
            import jax

            jax.devices()
            if device_ids:
                ids = (ctypes.c_int64 * len(device_ids))(*device_ids)
                rc = lib.axon_start_nrt_profile(ids, len(device_ids))
            else:
                rc = lib.axon_start_nrt_profile(None, 0)
            if rc != 0:
                raise RuntimeError(f"axon_start_nrt_profile rc={rc}")
            try:
                yield
            finally:
                n = lib.axon_stop_nrt_profile(str(output_dir).encode())
                print(f"profile: {n} file(s) written to {output_dir}")

        mod._hook = _hook
    except (OSError, AttributeError):
        pass

    import antenv

    _sys.modules["antenv.axon_hooks"] = mod
    antenv.axon_hooks = mod


def _exact_row_scores(features, memory_bank, rows, kk):
    """Exact numpy top-k mean distance for a few suspect rows."""
    f = features[rows]  # [R, D]
    d2 = (
        np.einsum("rd,rd->r", f, f)[:, None]
        + np.einsum("nd,nd->n", memory_bank, memory_bank)[None, :]
        - 2.0 * (f @ memory_bank.T)
    )
    d2k = np.sort(d2, axis=1)[:, :kk]
    return np.sqrt(np.maximum(d2k, 0.0)).mean(axis=1)


def kernel(features, memory_bank, k):
    global LAST_RESULT, N_RECOMPUTED
    from concourse.bass_utils import run_bass_kernel_spmd

    features = np.asarray(features, dtype=np.float32)
    memory_bank = np.asarray(memory_bank, dtype=np.float32)
    B, D = features.shape
    N = memory_bank.shape[0]
    kk = min(int(k), N)
    if kk <= 0:
        # mean over an empty candidate set (matches jnp.mean of empty)
        return np.full(B, np.nan, np.float32)

    in_maps, NPAD, x_sq, msq = _host_prep(features, memory_bank)
    nc = _build(B, D, NPAD)

    if TRACE:
        _install_ntff_hook()
    res = run_bass_kernel_spmd(nc, in_maps, list(range(NCORES)), trace=TRACE)
    LAST_RESULT = res

    # gather per-(core, block) top-8 candidates; larger v = closer
    MT = B // P
    v = np.concatenate(
        [_untile_cand(res.results[i]["cand"], MT) for i in range(NCORES)], axis=1
    )  # [B, NCORES * 8 * nblocks]
    return _finalize(v, x_sq, features, memory_bank, kk)


def _untile_cand(arr, MT):
    """Device cand layout [P, MT*CW] -> [B, CW] (b = m*P + p)."""
    CW = arr.shape[1] // MT
    return arr.reshape(P, MT, CW).transpose(1, 0, 2).reshape(MT * P, CW)


def _finalize(v, x_sq, features, memory_bank, kk):
    """Reduce the per-(core, block) top-8 candidates to the final scores."""
    global N_RECOMPUTED
    kk_c = min(kk, v.shape[1])
    order = np.argsort(-v, axis=1)[:, :kk_c]  # observed top-k candidates
    vk = np.take_along_axis(v, order, axis=1)
    # v = f.m_trunc + C_M - |m|^2/2  =>  d^2 = x_sq + 2*C_M - 2*v
    d = np.sqrt(np.maximum(x_sq[:, None] + 2.0 * C_M - 2.0 * vk, 0.0))
    scores = d.mean(axis=1).astype(np.float32)

    # A true top-k member can only be missing if >=8 elements of its
    # 1024-column block outrank it; then >=8 of the observed top-k come
    # from that block (index group of 8).  Recompute such rows exactly.
    N_RECOMPUTED = 0
    if kk >= 9:
        if kk > v.shape[1]:  # more than the candidate pool: all rows exact
            suspects = np.arange(v.shape[0])
        else:
            grp = np.sort(order // 8, axis=1)
            same8 = (grp[:, 7:] == grp[:, : grp.shape[1] - 7]).any(axis=1)
            suspects = np.nonzero(same8)[0]
        if suspects.size:
            N_RECOMPUTED = suspects.size
            scores[suspects] = _exact_row_scores(
                features, memory_bank, suspects, kk
            ).astype(np.float32)

    return scores

